# revision 34
# baseline (speedup 1.0000x reference)
"""Trainium2 Bass kernel for nn_DiagonalTraining (anti-diagonal per-diag Linear).

out[b, r, c] = sum_{r'} W[d, r - r0(d), r' - r0(d)] * x[b, r', d - r'] + bias,
with d = r + c over the valid range of r' for diagonal d.

v2 strategy (v1 was f32 + heavy padding, 20.1 MB DMA/core, 71.7 us, fully
DMA-bound): cut DMA bytes ~2.7x.

- All device traffic is bf16 (inputs, weights, outputs; PSUM accumulates f32).
  Measured numpy rel-err 2.9e-3 vs the 2e-2 gate.
- Diagonal lengths are padded up to multiples of 4. Each length-class
  (n' = 4j, j=1..64) contains exactly 8 diagonals (class 256 gets 1 dummy),
  so dealing one member per core gives every core the SAME shape schedule:
  required for the single SPMD program.
- Per core: 32 "pairs", each = one long diag (n' = 132..256, two K-chunks
  PSUM-accumulated) + one short diag (n' = 128..4, one chunk), sharing a
  260-col PSUM region in bank s%8.
- K-chunks sit at 32-aligned SBUF partition offsets (PE quadrant grid rule:
  K<=32 at 0/32/64/96, K<=64 at 0/64, else 0). The host bin-packs the
  xd [K,128] and W [K,n'] tiles of partial chunks into shared 128-partition
  rectangles, so the flat group DMAs move almost no padding.
- PSUM->SBUF copies (f32->bf16) alternate vector/scalar in tapered blocks;
  scalar self-issues its blocks' output DMAs, sync issues vector's (DVE
  cannot start DMAs; GPSIMD cannot access PSUM). Input streams in 9 group
  DMAs from sync (tiny first group so the PE starts ASAP); warmup matmuls
  on garbage SBUF keep the PE busy through engine boot so the HAM clock
  gate is released when real work arrives.

Measured on trn2 (8 cores): 71.7us (v1) -> 35.1-35.4us, rel err 2.9e-3.
The remaining time is ~7.5us engine boot + ~3.5us first-DMA latency +
~22us HBM-bound window (8.1 MB/core at ~370 GB/s) + ~2.5us tail.
"""

import sys

sys.path.insert(0, "/opt/trn_rl_repo")

import numpy as np

B, S = 128, 256
D = 2 * S - 1  # 511
NCORES = 8
GRAN = 4
NCLS = 64
NPAIRS = 32
NGROUPS = 9  # len(GROUP_SIZES)
NWARM = 14  # PE warmup matmuls (252 rows each) during engine boot
PAIR_W = 260  # n'L + n'S, constant across pairs
YCOLS = NPAIRS * PAIR_W  # 8320

USE_BF16 = True  # kept for test.py compat; v2 is always bf16
TRACE = False
last_results = None


def _geom(d):
    r0 = max(0, d - S + 1)
    n = d + 1 if d < S else 2 * S - 1 - d
    return r0, n


def _ceil32(k):
    return ((k + 31) // 32) * 32


def _classes():
    """class j (1..64): diagonals with n in (4j-4, 4j]; each has 8 members
    (class 64: 7 real + 1 dummy None)."""
    cls = [[] for _ in range(NCLS + 1)]
    for d in range(D):
        _, n = _geom(d)
        cls[(n + GRAN - 1) // GRAN].append(d)
    cls[NCLS].append(None)
    for j in range(1, NCLS + 1):
        assert len(cls[j]) == 8, (j, len(cls[j]))
    return cls


def _build_layout():
    """Shape-level schedule + column layout, identical for all cores.

    Returns dict:
      pairs[s] = (jL, jS)
      chunks[s] = list of (tag, k0, K, w, pbase, xcol, wcol)
                  tag in {L1, L2, SS}; AP partition base pbase; xd tile at
                  IN[pbase:pbase+K, xcol:xcol+128]; W tile at
                  IN[pbase:pbase+K, wcol:wcol+w].
      groups = [(c0, c1)] per input DMA group
      gid[s] = input group of pair s
      Lin
    """
    # schedule: smallest pairs at BOTH ends (fast first compute after the
    # first small group; short tail after the last group), biggest mid-run.
    base = [(33 + i, 32 - i) for i in range(NPAIRS)]  # ascending W bytes
    lo, hi = [], []
    for i in range(NPAIRS):
        (lo if i % 2 == 0 else hi).append(i)
    order = lo + hi[::-1]  # 0,2,4,..,30,31,29,..,1
    pairs = [base[i] for i in order]
    chunks = [None] * NPAIRS
    groups = []
    gid = []
    cur = 0
    s0 = 0
    for g, gsz in enumerate(GROUP_SIZES):
        c0 = cur
        full = []  # (s, tag, k0, w)
        part = []  # (s, tag, k0, K, w)
        for s in range(s0, s0 + gsz):
            gid.append(g)
            jL, jS = pairs[s]
            nL, nS = GRAN * jL, GRAN * jS
            full.append((s, "L1", 0, nL))
            part.append((s, "L2", 128, nL - 128, nL))
            part.append((s, "SS", 0, nS, nS))
            chunks[s] = []
        placed = {}  # (s, tag) -> (k0, K, w, pbase, xcol, wcol)
        for s, tag, k0, w in full:
            xcol = cur
            cur += 128
            wcol = cur
            cur += w
            placed[(s, tag)] = (k0, 128, w, 0, xcol, wcol)
        # first-fit-decreasing bin pack of partial chunks onto 4 strips.
        # AP base partition must be 0/32/64 (quadrant grid, 96 rejected by
        # bass), so strip 3 is only reachable as the tail of a >=2-strip
        # placement.
        bins = []  # dict(free=[bool]*4, wmax, members=[(s,tag,k0,K,w,pbase)])
        BASES = {4: [0], 3: [0], 2: [0, 2], 1: [0, 1, 2]}
        for s, tag, k0, K, w in sorted(
            part, key=lambda it: (-_ceil32(it[3]), -it[4])
        ):
            ns = _ceil32(K) // 32
            # best-fit: pick the (bin, base) minimizing W-width growth
            best = None
            for bn in bins:
                for ba in BASES[ns]:
                    if all(bn["free"][ba : ba + ns]):
                        grow = max(bn["wmax"], w) - bn["wmax"]
                        if best is None or grow < best[0]:
                            best = (grow, bn, ba)
                        break
            done = best is not None
            if done:
                _, bn, ba = best
                for q in range(ba, ba + ns):
                    bn["free"][q] = False
                bn["wmax"] = max(bn["wmax"], w)
                bn["members"].append((s, tag, k0, K, w, ba * 32))
            if not done:
                bn = dict(free=[True] * 4, wmax=w, members=[])
                for q in range(ns):
                    bn["free"][q] = False
                bn["members"].append((s, tag, k0, K, w, 0))
                bins.append(bn)
        for bn in bins:
            xcol = cur
            cur += 128
            wcol = cur
            cur += bn["wmax"]
            for s, tag, k0, K, w, pbase in bn["members"]:
                placed[(s, tag)] = (k0, K, w, pbase, xcol, wcol)
        for s in range(s0, s0 + gsz):
            for tag in ("L1", "L2", "SS"):
                k0, K, w, pbase, xcol, wcol = placed[(s, tag)]
                chunks[s].append((tag, k0, K, w, pbase, xcol, wcol))
        groups.append((c0, cur))
        s0 += gsz
    return dict(pairs=pairs, chunks=chunks, groups=groups, gid=gid, Lin=cur)


# Input DMA groups: tiny first group so compute starts right after boot.
GROUP_SIZES = [1, 3, 4, 4, 4, 4, 4, 4, 4]
assert sum(GROUP_SIZES) == NPAIRS

# copy/output-DMA blocks: (engine, n_pairs), consecutive schedule slots.
# GPSIMD cannot access PSUM and DVE cannot issue DMAs: copies alternate
# vector/scalar; scalar self-issues its output DMAs, sync issues vector's.
# Tiny last block shrinks the tail.
BLOCKS = [("vector", 5), ("scalar", 5), ("vector", 5), ("scalar", 5),
          ("vector", 4), ("scalar", 4), ("vector", 3), ("scalar", 1)]
assert sum(n for _, n in BLOCKS) == NPAIRS


def _block_ranges():
    out = []
    s0 = 0
    for eng, n in BLOCKS:
        out.append((eng, s0, s0 + n))
        s0 += n
    return out


def _eng_pairs(eng):
    out = []
    for e, a, b in _block_ranges():
        if e == eng:
            out.extend(range(a, b))
    return out


_TABLES = None
_PROG = None


def _tables():
    global _TABLES
    if _TABLES is None:
        layout = _build_layout()
        cls = _classes()
        # per-core diag assignment + scatter targets
        cores = []
        for c in range(NCORES):
            jobs = []  # per pair: (dL, dS)
            tgt = np.full(YCOLS, -1, np.int64)
            for s, (jL, jS) in enumerate(layout["pairs"]):
                dL = cls[jL][c]
                dS = cls[jS][c]
                jobs.append((dL, dS))
                y0 = s * PAIR_W
                for d, off, wpad in ((dL, 0, GRAN * jL), (dS, GRAN * jL, GRAN * jS)):
                    if d is None:
                        continue
                    r0, n = _geom(d)
                    m = np.arange(n)
                    tgt[y0 + off + m] = (r0 + m) * S + (d - r0 - m)
            cores.append(dict(jobs=jobs, tgt=tgt))
        # bias gather: out_flat[p] += b[d, r - r0(d)], p = r*S+c, d = r+c
        rr, cc = np.divmod(np.arange(S * S), S)
        dd = rr + cc
        r0v = np.maximum(0, dd - S + 1)
        bidx = dd * S + (rr - r0v)
        _TABLES = (layout, cores, bidx)
    return _TABLES


def _build_program():
    import concourse.bass as bass
    import concourse.mybir as mybir

    layout, cores, _ = _tables()
    Lin = layout["Lin"]
    f32 = mybir.dt.float32
    bf16 = mybir.dt.bfloat16

    nc = bass.Bass()
    bi = nc.dram_tensor("bi", [128, Lin], bf16, kind="ExternalInput")
    y = nc.dram_tensor("y", [128, YCOLS], bf16, kind="ExternalOutput")

    IN = nc.alloc_sbuf_tensor("IN", [128, Lin], bf16).ap()
    Y = nc.alloc_sbuf_tensor("Y", [128, YCOLS], bf16).ap()
    WU = nc.alloc_sbuf_tensor("WU", [128, 256], bf16).ap()  # never written
    PS = [nc.alloc_psum_tensor(f"ps{i}", [128, 512], f32).ap() for i in range(8)]

    DIN = [nc.alloc_semaphore(f"din{g}") for g in range(NGROUPS)]
    P = nc.alloc_semaphore("P")
    CC = {e: nc.alloc_semaphore(f"C{e[0]}") for e in ("vector", "scalar")}
    DO = nc.alloc_semaphore("DO")

    eng_pairs = {e: _eng_pairs(e) for e in ("vector", "scalar")}

    def pair_region(s):
        jL, jS = layout["pairs"][s]
        return GRAN * jL, GRAN * jS  # widths (long, short)

    with nc.Block(no_gpsimd_drain=True) as block:

        @block.sync
        def _(sync):
            for g, (c0, c1) in enumerate(layout["groups"]):
                sync.dma_start(out=IN[:, c0:c1], in_=bi[:, c0:c1]).then_inc(
                    DIN[g], 16
                )
            # DVE cannot issue DMAs; sync drains the vector blocks' outputs
            # (SP HWDGE, known-good 16-inc completion semantics).
            nv = 0
            for e, a, b in _block_ranges():
                if e != "vector":
                    continue
                nv += b - a
                sync.wait_ge(CC["vector"], nv)
                sync.dma_start(
                    out=y[:, a * PAIR_W : b * PAIR_W],
                    in_=Y[:, a * PAIR_W : b * PAIR_W],
                ).then_inc(DO, 16)
            sync.wait_ge(DO, 16 * len(BLOCKS))

        def block_of(u):
            for e, a, b in _block_ranges():
                if a <= u < b:
                    return e
            raise AssertionError(u)

        @block.tensor
        def _(tensor):
            # warmup: keep the PE continuously busy through the boot window
            # so DVFS has it at full clock when real matmuls start. Reads
            # uninitialized SBUF (finite-or-NaN garbage) into a PSUM region
            # no real pair touches; never read back.
            for _i in range(NWARM):
                nc.tensor.matmul(
                    PS[7][:, 260:512], WU[:, 0:128], WU[:, 0:252],
                    start=True, stop=True,
                )
            cur_g = -1
            for s in range(NPAIRS):
                g = layout["gid"][s]
                if g > cur_g:
                    tensor.wait_ge(DIN[g], 16)
                    cur_g = g
                if s >= 8:
                    u = s - 8
                    e = block_of(u)
                    thr = eng_pairs[e].index(u) + 1
                    tensor.wait_ge(CC[e], thr)
                nL, nS = pair_region(s)
                ps = PS[s % 8]
                mm = None
                for tag, k0, K, w, pbase, xcol, wcol in layout["chunks"][s]:
                    lhsT = IN[pbase : pbase + K, xcol : xcol + 128]
                    rhs = IN[pbase : pbase + K, wcol : wcol + w]
                    if tag == "L1":
                        out = ps[:, 0:nL]
                        st, sp = True, False
                    elif tag == "L2":
                        out = ps[:, 0:nL]
                        st, sp = False, True
                    else:
                        out = ps[:, nL : nL + nS]
                        st, sp = True, True
                    mm = nc.tensor.matmul(out, lhsT, rhs, start=st, stop=sp)
                mm.then_inc(P, 1)

        def copy_body(eng_name, eng_ops):
            copier = (
                nc.scalar.copy if eng_name == "scalar" else nc.vector.tensor_copy
            )
            for e, a, b in _block_ranges():
                if e != eng_name:
                    continue
                for s in range(a, b):
                    eng_ops.wait_ge(P, s + 1)
                    cp = copier(
                        Y[:, s * PAIR_W : (s + 1) * PAIR_W],
                        PS[s % 8][:, 0:PAIR_W],
                    )
                    cp.then_inc(CC[eng_name], 1)
                if eng_name != "vector":  # DVE cannot issue DMAs
                    eng_ops.dma_start(
                        out=y[:, a * PAIR_W : b * PAIR_W],
                        in_=Y[:, a * PAIR_W : b * PAIR_W],
                    ).then_inc(DO, 16)

        @block.vector
        def _(vector):
            copy_body("vector", vector)

        @block.scalar
        def _(scalar):
            copy_body("scalar", scalar)



    return nc


def _get_program():
    global _PROG
    if _PROG is None:
        _PROG = _build_program()
    return _PROG


def _pack_core(core, layout, x, W, np_bf16):
    IN = np.zeros((128, layout["Lin"]), np_bf16)
    for s, (dL, dS) in enumerate(core["jobs"]):
        for tag, k0, K, w, pbase, xcol, wcol in layout["chunks"][s]:
            d = dS if tag == "SS" else dL
            if d is None:
                continue
            r0, n = _geom(d)
            kn = min(n - k0, K)
            if kn > 0:
                kv = np.arange(k0, k0 + kn)
                IN[pbase : pbase + kn, xcol : xcol + 128] = x[
                    :, r0 + kv, d - r0 - kv
                ].T
            # rhs[k, m] = W[d, m, k0+k]; W is zero beyond [n, n] so padding
            # contributes exactly zero.
            IN[pbase : pbase + K, wcol : wcol + w] = W[d, 0:w, k0 : k0 + K].T
    return {"bi": IN}


def kernel(x, W, b):
    import ml_dtypes
    from concourse.bass_utils import run_bass_kernel_spmd

    x = np.asarray(x, np.float32)
    W = np.asarray(W, np.float32)
    b = np.asarray(b, np.float32)
    layout, cores, bidx = _tables()
    in_maps = [
        _pack_core(core, layout, x, W, ml_dtypes.bfloat16) for core in cores
    ]
    nc = _get_program()
    res = run_bass_kernel_spmd(
        nc, in_maps, core_ids=list(range(NCORES)), trace=TRACE
    )
    global last_results
    last_results = res
    out_flat = np.zeros((B, S * S), np.float32)
    for c, core in enumerate(cores):
        Yc = np.asarray(res.results[c]["y"]).astype(np.float32)
        tgt = core["tgt"]
        v = tgt >= 0
        out_flat[:, tgt[v]] = Yc[:, v]
    out_flat += b.reshape(-1)[bidx][None, :]
    return out_flat.reshape(B, S, S)


# revision 35
# speedup vs baseline: 1.0576x; 1.0576x over previous
"""Trainium2 Bass kernel for nn_DiagonalTraining (anti-diagonal per-diag Linear).

out[b, r, c] = sum_{r'} W[d, r - r0(d), r' - r0(d)] * x[b, r', d - r'] + bias,
with d = r + c over the valid range of r' for diagonal d.

v2 strategy (v1 was f32 + heavy padding, 20.1 MB DMA/core, 71.7 us, fully
DMA-bound): cut DMA bytes ~2.7x.

- All device traffic is bf16 (inputs, weights, outputs; PSUM accumulates f32).
  Measured numpy rel-err 2.9e-3 vs the 2e-2 gate.
- Diagonal lengths are padded up to multiples of 4. Each length-class
  (n' = 4j, j=1..64) contains exactly 8 diagonals (class 256 gets 1 dummy),
  so dealing one member per core gives every core the SAME shape schedule:
  required for the single SPMD program.
- Per core: 32 "pairs", each = one long diag (n' = 132..256, two K-chunks
  PSUM-accumulated) + one short diag (n' = 128..4, one chunk), sharing a
  260-col PSUM region in bank s%8.
- K-chunks sit at 32-aligned SBUF partition offsets (PE quadrant grid rule:
  K<=32 at 0/32/64/96, K<=64 at 0/64, else 0). The host bin-packs the
  xd [K,128] and W [K,n'] tiles of partial chunks into shared 128-partition
  rectangles, so the flat group DMAs move almost no padding.
- PSUM->SBUF copies (f32->bf16) alternate vector/scalar in tapered blocks;
  scalar self-issues its blocks' output DMAs, sync issues vector's (DVE
  cannot start DMAs; GPSIMD cannot access PSUM). Input streams in 9 group
  DMAs from sync (tiny first group so the PE starts ASAP); warmup matmuls
  on garbage SBUF keep the PE busy through engine boot so the HAM clock
  gate is released when real work arrives.

Measured on trn2 (8 cores): 71.7us (v1) -> 35.1-37.0us run-to-run,
rel err 2.9e-3. Breakdown: ~7.5us engine boot (program loads + runtime
init; +~1.4us if any scalar activation op pulls the ACT table) + ~3.5us
first-DMA DGE latency + ~18.5us HBM-saturated window (8.1 MB/core, DMA
engines 16/16 busy) + ~5us tail where the clock-throttled PE (HAM grants
2.4 GHz for one 3.4us window, then holds K=4/8 = 1.2 GHz under sustained
8-core load) drains its backlog.

Measured dead ends, do not retry blindly:
- fp32r padded K=128 bins (v1): DMA-bound at 2.5x the bytes.
- PE warmup to beat the clock gate: HAM re-throttles after one window.
- All-vector copies (no scalar): boot shrinks but the serial copy chain
  gates PSUM bank reuse; net +2us.
- Two 256-col pair regions per PSUM bank (16 slots, jL+jS=64 pairing):
  compiles + passes numpy emulation but faults the device - concurrent
  PE-write/DVE-read on a shared bank appears illegal.
- Bin-packing pooling across DMA groups: saves <0.1 MB; the ~1 MB pack
  overhead is PE-quadrant-alignment structural, not bin underfill.
"""

import sys

sys.path.insert(0, "/opt/trn_rl_repo")

import numpy as np

B, S = 128, 256
D = 2 * S - 1  # 511
NCORES = 8
GRAN = 4
NCLS = 64
NPAIRS = 32
NGROUPS = 9  # len(GROUP_SIZES)
NWARM = 14  # PE warmup matmuls (252 rows each) during engine boot
PAIR_W = 260  # n'L + n'S, constant across pairs
YCOLS = NPAIRS * PAIR_W  # 8320

USE_BF16 = True  # kept for test.py compat; v2 is always bf16
TRACE = False
last_results = None


def _geom(d):
    r0 = max(0, d - S + 1)
    n = d + 1 if d < S else 2 * S - 1 - d
    return r0, n


def _ceil32(k):
    return ((k + 31) // 32) * 32


def _classes():
    """class j (1..64): diagonals with n in (4j-4, 4j]; each has 8 members
    (class 64: 7 real + 1 dummy None)."""
    cls = [[] for _ in range(NCLS + 1)]
    for d in range(D):
        _, n = _geom(d)
        cls[(n + GRAN - 1) // GRAN].append(d)
    cls[NCLS].append(None)
    for j in range(1, NCLS + 1):
        assert len(cls[j]) == 8, (j, len(cls[j]))
    return cls


def _build_layout():
    """Shape-level schedule + column layout, identical for all cores.

    Returns dict:
      pairs[s] = (jL, jS)
      chunks[s] = list of (tag, k0, K, w, pbase, xcol, wcol)
                  tag in {L1, L2, SS}; AP partition base pbase; xd tile at
                  IN[pbase:pbase+K, xcol:xcol+128]; W tile at
                  IN[pbase:pbase+K, wcol:wcol+w].
      groups = [(c0, c1)] per input DMA group
      gid[s] = input group of pair s
      Lin
    """
    # schedule: smallest pairs at BOTH ends (fast first compute after the
    # first small group; short tail after the last group), biggest mid-run.
    base = [(33 + i, 32 - i) for i in range(NPAIRS)]  # ascending W bytes
    lo, hi = [], []
    for i in range(NPAIRS):
        (lo if i % 2 == 0 else hi).append(i)
    order = lo + hi[::-1]  # 0,2,4,..,30,31,29,..,1
    pairs = [base[i] for i in order]
    chunks = [None] * NPAIRS
    groups = []
    gid = []
    cur = 0
    s0 = 0
    for g, gsz in enumerate(GROUP_SIZES):
        c0 = cur
        full = []  # (s, tag, k0, w)
        part = []  # (s, tag, k0, K, w)
        for s in range(s0, s0 + gsz):
            gid.append(g)
            jL, jS = pairs[s]
            nL, nS = GRAN * jL, GRAN * jS
            full.append((s, "L1", 0, nL))
            part.append((s, "L2", 128, nL - 128, nL))
            part.append((s, "SS", 0, nS, nS))
            chunks[s] = []
        placed = {}  # (s, tag) -> (k0, K, w, pbase, xcol, wcol)
        for s, tag, k0, w in full:
            xcol = cur
            cur += 128
            wcol = cur
            cur += w
            placed[(s, tag)] = (k0, 128, w, 0, xcol, wcol)
        # first-fit-decreasing bin pack of partial chunks onto 4 strips.
        # AP base partition must be 0/32/64 (quadrant grid, 96 rejected by
        # bass), so strip 3 is only reachable as the tail of a >=2-strip
        # placement.
        bins = []  # dict(free=[bool]*4, wmax, members=[(s,tag,k0,K,w,pbase)])
        BASES = {4: [0], 3: [0], 2: [0, 2], 1: [0, 1, 2]}
        for s, tag, k0, K, w in sorted(
            part, key=lambda it: (-_ceil32(it[3]), -it[4])
        ):
            ns = _ceil32(K) // 32
            # best-fit: pick the (bin, base) minimizing W-width growth
            best = None
            for bn in bins:
                for ba in BASES[ns]:
                    if all(bn["free"][ba : ba + ns]):
                        grow = max(bn["wmax"], w) - bn["wmax"]
                        if best is None or grow < best[0]:
                            best = (grow, bn, ba)
                        break
            done = best is not None
            if done:
                _, bn, ba = best
                for q in range(ba, ba + ns):
                    bn["free"][q] = False
                bn["wmax"] = max(bn["wmax"], w)
                bn["members"].append((s, tag, k0, K, w, ba * 32))
            if not done:
                bn = dict(free=[True] * 4, wmax=w, members=[])
                for q in range(ns):
                    bn["free"][q] = False
                bn["members"].append((s, tag, k0, K, w, 0))
                bins.append(bn)
        for bn in bins:
            xcol = cur
            cur += 128
            wcol = cur
            cur += bn["wmax"]
            for s, tag, k0, K, w, pbase in bn["members"]:
                placed[(s, tag)] = (k0, K, w, pbase, xcol, wcol)
        for s in range(s0, s0 + gsz):
            for tag in ("L1", "L2", "SS"):
                k0, K, w, pbase, xcol, wcol = placed[(s, tag)]
                chunks[s].append((tag, k0, K, w, pbase, xcol, wcol))
        groups.append((c0, cur))
        s0 += gsz
    return dict(pairs=pairs, chunks=chunks, groups=groups, gid=gid, Lin=cur)


# Input DMA groups: tiny first group so compute starts right after boot.
GROUP_SIZES = [1, 3, 4, 4, 4, 4, 4, 4, 4]
assert sum(GROUP_SIZES) == NPAIRS

# copy/output-DMA blocks: (engine, n_pairs), consecutive schedule slots.
# GPSIMD cannot access PSUM and DVE cannot issue DMAs: copies alternate
# vector/scalar; scalar self-issues its output DMAs, sync issues vector's.
# Tiny last block shrinks the tail.
BLOCKS = [("vector", 5), ("scalar", 5), ("vector", 5), ("scalar", 5),
          ("vector", 4), ("scalar", 4), ("vector", 3), ("scalar", 1)]
assert sum(n for _, n in BLOCKS) == NPAIRS


def _block_ranges():
    out = []
    s0 = 0
    for eng, n in BLOCKS:
        out.append((eng, s0, s0 + n))
        s0 += n
    return out


def _eng_pairs(eng):
    out = []
    for e, a, b in _block_ranges():
        if e == eng:
            out.extend(range(a, b))
    return out


_TABLES = None
_PROG = None


def _tables():
    global _TABLES
    if _TABLES is None:
        layout = _build_layout()
        cls = _classes()
        # per-core diag assignment + scatter targets
        cores = []
        for c in range(NCORES):
            jobs = []  # per pair: (dL, dS)
            tgt = np.full(YCOLS, -1, np.int64)
            for s, (jL, jS) in enumerate(layout["pairs"]):
                dL = cls[jL][c]
                dS = cls[jS][c]
                jobs.append((dL, dS))
                y0 = s * PAIR_W
                for d, off, wpad in ((dL, 0, GRAN * jL), (dS, GRAN * jL, GRAN * jS)):
                    if d is None:
                        continue
                    r0, n = _geom(d)
                    m = np.arange(n)
                    tgt[y0 + off + m] = (r0 + m) * S + (d - r0 - m)
            cores.append(dict(jobs=jobs, tgt=tgt))
        # bias gather: out_flat[p] += b[d, r - r0(d)], p = r*S+c, d = r+c
        rr, cc = np.divmod(np.arange(S * S), S)
        dd = rr + cc
        r0v = np.maximum(0, dd - S + 1)
        bidx = dd * S + (rr - r0v)
        _TABLES = (layout, cores, bidx)
    return _TABLES


def _build_program():
    import concourse.bass as bass
    import concourse.mybir as mybir

    layout, cores, _ = _tables()
    Lin = layout["Lin"]
    f32 = mybir.dt.float32
    bf16 = mybir.dt.bfloat16

    nc = bass.Bass()
    bi = nc.dram_tensor("bi", [128, Lin], bf16, kind="ExternalInput")
    y = nc.dram_tensor("y", [128, YCOLS], bf16, kind="ExternalOutput")

    IN = nc.alloc_sbuf_tensor("IN", [128, Lin], bf16).ap()
    Y = nc.alloc_sbuf_tensor("Y", [128, YCOLS], bf16).ap()
    WU = nc.alloc_sbuf_tensor("WU", [128, 256], bf16).ap()  # never written
    PS = [nc.alloc_psum_tensor(f"ps{i}", [128, 512], f32).ap() for i in range(8)]

    DIN = [nc.alloc_semaphore(f"din{g}") for g in range(NGROUPS)]
    P = nc.alloc_semaphore("P")
    CC = {e: nc.alloc_semaphore(f"C{e[0]}") for e in ("vector", "scalar")}
    DO = nc.alloc_semaphore("DO")

    eng_pairs = {e: _eng_pairs(e) for e in ("vector", "scalar")}

    def pair_region(s):
        jL, jS = layout["pairs"][s]
        return GRAN * jL, GRAN * jS  # widths (long, short)

    with nc.Block(no_gpsimd_drain=True) as block:

        @block.sync
        def _(sync):
            for g, (c0, c1) in enumerate(layout["groups"]):
                sync.dma_start(out=IN[:, c0:c1], in_=bi[:, c0:c1]).then_inc(
                    DIN[g], 16
                )
            # DVE cannot issue DMAs; sync drains the vector blocks' outputs
            # (SP HWDGE, known-good 16-inc completion semantics).
            nv = 0
            for e, a, b in _block_ranges():
                if e != "vector":
                    continue
                nv += b - a
                sync.wait_ge(CC["vector"], nv)
                sync.dma_start(
                    out=y[:, a * PAIR_W : b * PAIR_W],
                    in_=Y[:, a * PAIR_W : b * PAIR_W],
                ).then_inc(DO, 16)
            sync.wait_ge(DO, 16 * len(BLOCKS))

        def block_of(u):
            for e, a, b in _block_ranges():
                if a <= u < b:
                    return e
            raise AssertionError(u)

        @block.tensor
        def _(tensor):
            # warmup: keep the PE continuously busy through the boot window
            # so DVFS has it at full clock when real matmuls start. Reads
            # uninitialized SBUF (finite-or-NaN garbage) into a PSUM region
            # no real pair touches; never read back.
            for _i in range(NWARM):
                nc.tensor.matmul(
                    PS[7][:, 260:512], WU[:, 0:128], WU[:, 0:252],
                    start=True, stop=True,
                )
            cur_g = -1
            for s in range(NPAIRS):
                g = layout["gid"][s]
                if g > cur_g:
                    tensor.wait_ge(DIN[g], 16)
                    cur_g = g
                if s >= 8:
                    u = s - 8
                    e = block_of(u)
                    thr = eng_pairs[e].index(u) + 1
                    tensor.wait_ge(CC[e], thr)
                nL, nS = pair_region(s)
                ps = PS[s % 8]
                mm = None
                for tag, k0, K, w, pbase, xcol, wcol in layout["chunks"][s]:
                    lhsT = IN[pbase : pbase + K, xcol : xcol + 128]
                    rhs = IN[pbase : pbase + K, wcol : wcol + w]
                    if tag == "L1":
                        out = ps[:, 0:nL]
                        st, sp = True, False
                    elif tag == "L2":
                        out = ps[:, 0:nL]
                        st, sp = False, True
                    else:
                        out = ps[:, nL : nL + nS]
                        st, sp = True, True
                    mm = nc.tensor.matmul(out, lhsT, rhs, start=st, stop=sp)
                mm.then_inc(P, 1)

        def copy_body(eng_name, eng_ops):
            copier = (
                nc.scalar.copy if eng_name == "scalar" else nc.vector.tensor_copy
            )
            for e, a, b in _block_ranges():
                if e != eng_name:
                    continue
                for s in range(a, b):
                    eng_ops.wait_ge(P, s + 1)
                    cp = copier(
                        Y[:, s * PAIR_W : (s + 1) * PAIR_W],
                        PS[s % 8][:, 0:PAIR_W],
                    )
                    cp.then_inc(CC[eng_name], 1)
                if eng_name != "vector":  # DVE cannot issue DMAs
                    eng_ops.dma_start(
                        out=y[:, a * PAIR_W : b * PAIR_W],
                        in_=Y[:, a * PAIR_W : b * PAIR_W],
                    ).then_inc(DO, 16)

        @block.vector
        def _(vector):
            copy_body("vector", vector)

        @block.scalar
        def _(scalar):
            copy_body("scalar", scalar)



    return nc


def _get_program():
    global _PROG
    if _PROG is None:
        _PROG = _build_program()
    return _PROG


def _pack_core(core, layout, x, W, np_bf16):
    IN = np.zeros((128, layout["Lin"]), np_bf16)
    for s, (dL, dS) in enumerate(core["jobs"]):
        for tag, k0, K, w, pbase, xcol, wcol in layout["chunks"][s]:
            d = dS if tag == "SS" else dL
            if d is None:
                continue
            r0, n = _geom(d)
            kn = min(n - k0, K)
            if kn > 0:
                kv = np.arange(k0, k0 + kn)
                IN[pbase : pbase + kn, xcol : xcol + 128] = x[
                    :, r0 + kv, d - r0 - kv
                ].T
            # rhs[k, m] = W[d, m, k0+k]; W is zero beyond [n, n] so padding
            # contributes exactly zero.
            IN[pbase : pbase + K, wcol : wcol + w] = W[d, 0:w, k0 : k0 + K].T
    return {"bi": IN}


def kernel(x, W, b):
    import ml_dtypes
    from concourse.bass_utils import run_bass_kernel_spmd

    x = np.asarray(x, np.float32)
    W = np.asarray(W, np.float32)
    b = np.asarray(b, np.float32)
    layout, cores, bidx = _tables()
    in_maps = [
        _pack_core(core, layout, x, W, ml_dtypes.bfloat16) for core in cores
    ]
    nc = _get_program()
    res = run_bass_kernel_spmd(
        nc, in_maps, core_ids=list(range(NCORES)), trace=TRACE
    )
    global last_results
    last_results = res
    out_flat = np.zeros((B, S * S), np.float32)
    for c, core in enumerate(cores):
        Yc = np.asarray(res.results[c]["y"]).astype(np.float32)
        tgt = core["tgt"]
        v = tgt >= 0
        out_flat[:, tgt[v]] = Yc[:, v]
    out_flat += b.reshape(-1)[bidx][None, :]
    return out_flat.reshape(B, S, S)
